# revision 3
# baseline (speedup 1.0000x reference)
"""MoE (8 experts, top-2) expert-parallel Trainium2 kernel, v2.

Contract: kernel(**inputs) takes the full unsharded inputs and returns the
full [8, 2048, 768] output.  Internally:
  - host computes the gate (scores -> top-2 -> softmax) in float64 and
    dispatches tokens to experts (the "all-to-all" of the sharding hint),
  - each of the 8 NeuronCores runs one expert's 3-layer GELU MLP over its
    routed tokens (padded to a common capacity C) via a Bass/Tile kernel,
  - host combines expert outputs with the gate weights.

v2 changes vs v1:
  - all matmul operands in bf16 (fp32 PSUM accumulation): same PE rate as
    f32r but half the DMA/SBUF traffic and FWL-fast weight loads;
    end-to-end error ~4e-3, well inside the 2e-2 gate.
  - weight "arenas": one contiguous SBUF tile + one DMA per weight block
    (per mb for W1, per jj for W2/W3) instead of many small pool tiles, so
    buffer-reuse WAR waits target only accumulation-group stop matmuls.
  - all token sub-splits are 512 wide (one short tail), minimizing matmul
    instruction count at the PSUM-bank limit.
  - a post-schedule IR pass drops engine-semaphore increments that no wait
    references and renumbers the rest (engines complete in FIFO order, so
    `sem >= v` means "the v-th ticking instruction completed"; unwaited
    ticks are pure overhead, ~15ns each on the PE queue).
"""

import os
import sys
import types

import numpy as np
import ml_dtypes

import concourse.bass as bass  # noqa: F401  (bass must import before mybir use)
import concourse.mybir as mybir
from concourse import bacc
from concourse.tile import TileContext
from concourse.bass_utils import run_bass_kernel_spmd

EMB, HID, HID2 = 768, 3072, 6144
NE, TOPK = 8, 2
P = 128   # partitions
WIN = 4   # layer-2 blocks per layer-3 PSUM accumulation window
K1, K2 = EMB // P, HID // P          # 6, 24 contraction tiles
MB1, MB2 = HID // P, HID2 // P       # 24, 48 output 128-blocks
J3 = EMB // P                        # 6 output blocks of layer 3


def _install_ntff_hook():
    """Make trace=True work when antenv.axon_hooks is missing in the image."""
    try:
        from antenv.axon_hooks import get_axon_ntff_profile_hook  # noqa: F401
        return
    except ImportError:
        pass
    try:
        from trn_agent_boot.trn_boot import _ntff_profile_via_ctypes
        hook = _ntff_profile_via_ctypes('/opt/axon/libaxon_pjrt.so')
        mod = types.ModuleType('antenv.axon_hooks')
        mod.get_axon_ntff_profile_hook = lambda: hook
        sys.modules['antenv.axon_hooks'] = mod
    except Exception:
        pass


# --------------------------------------------------------------------------
# Post-schedule semaphore strip (see module docstring).

def _strip_redundant_sem_incs(nc):
    insts = []
    for f in nc.m.functions:
        for bb in f.blocks:
            for inst in bb.instructions:
                insts.append((bb.name, inst))

    updaters, waiters, blockers = {}, {}, set()
    for bb_name, inst in insts:
        si = inst.sync_info
        if si is None:
            continue
        for u in (si.on_update or []):
            if u.sync_type != "semaphore":
                continue
            if not (u.update_mode == "sem-inc"
                    and (u.update_value in (None, 1))
                    and u.update_reg is None):
                blockers.add(u.id)
            updaters.setdefault(u.id, []).append((bb_name, inst, u))
        for w in (si.on_wait or []):
            if w.sync_type != "semaphore":
                continue
            if w.wait_mode != "sem-ge-imm" or w.wait_reg is not None:
                blockers.add(w.id)
            waiters.setdefault(w.id, []).append((inst, w))

    # Only strip sems whose every updater is a plain compute instruction on
    # one engine FIFO.  DMA instructions can fan out to multiple hardware
    # queues (completion is not FIFO w.r.t. one semaphore), so any sem a DMA
    # updates is left untouched.
    safe_types = ("InstMatmult", "InstActivation", "InstTensorTensor",
                  "InstCopy", "InstTensorReduce", "InstTensorScalarPtr")
    dropped = 0
    for sem_id, ups in updaters.items():
        if sem_id in blockers:
            continue
        if any(type(i).__name__ not in safe_types for _, i, _ in ups):
            continue
        if len({i.engine for _, i, _ in ups}) != 1 or len({b for b, _, _ in ups}) != 1:
            continue
        n = len(ups)
        wts = waiters.get(sem_id, [])
        vals = sorted({w.wait_value for _, w in wts})
        if vals and (vals[0] < 1 or vals[-1] > n):
            continue
        needed = set(vals)
        needed.add(n)  # keep the final tick
        keep = [i + 1 in needed for i in range(n)]
        if all(keep):
            continue
        new_rank, r = {}, 0
        for i in range(n):
            if keep[i]:
                r += 1
            new_rank[i + 1] = r
        for inst, w in wts:
            w.wait_value = new_rank[w.wait_value]
        for i, (_, inst, u) in enumerate(ups):
            if keep[i]:
                continue
            si = inst.sync_info
            inst.sync_info = mybir.SyncInfo(
                on_wait=list(si.on_wait or []),
                on_update=[x for x in si.on_update if x is not u],
            )
            dropped += 1
    return dropped


# --------------------------------------------------------------------------
# Device program.

def _subs_of(t):
    subs, o = [], 0
    while t - o > 512:
        subs.append((o, 512))
        o += 512
    subs.append((o, t - o))
    return subs


def _chunks_of(c):
    chunks = []
    rem = c
    while rem > 1536:
        chunks.append(1024)
        rem -= 1024
    chunks.append(rem)
    return chunks


def _build_program(C):
    f32 = mybir.dt.float32
    bf16 = mybir.dt.bfloat16
    GELU = mybir.ActivationFunctionType.Gelu
    IDENT = mybir.ActivationFunctionType.Identity

    nc = bacc.Bacc(None, target_bir_lowering=False)

    XT = nc.declare_dram_parameter("XT", [K1, P, C], bf16, isOutput=False)
    W1A = nc.declare_dram_parameter("W1A", [MB1, P, K1 * P], bf16, isOutput=False)
    W2A = nc.declare_dram_parameter("W2A", [MB2, P, K2 * P], bf16, isOutput=False)
    W3A = nc.declare_dram_parameter("W3A", [MB2, P, EMB], bf16, isOutput=False)
    B1 = nc.declare_dram_parameter("B1", [P, MB1], f32, isOutput=False)
    B2 = nc.declare_dram_parameter("B2", [P, MB2], f32, isOutput=False)
    B3 = nc.declare_dram_parameter("B3", [P, J3], f32, isOutput=False)
    YT = nc.declare_dram_parameter("YT", [J3, P, C], f32, isOutput=True)

    chunks = _chunks_of(C)
    max_t = max(chunks)

    with TileContext(nc) as tc:
        with (
            tc.tile_pool(name="bias", bufs=1) as bias_pool,
            tc.tile_pool(name="xt", bufs=1) as xt_pool,
            tc.tile_pool(name="h1", bufs=1) as h1_pool,
            tc.tile_pool(name="yac", bufs=1) as y_pool,
            tc.tile_pool(name="w1", bufs=3) as w1_pool,
            tc.tile_pool(name="w2", bufs=3) as w2_pool,
            tc.tile_pool(name="w3", bufs=6) as w3_pool,
            tc.tile_pool(name="h2", bufs=2) as h2_pool,
            tc.tile_pool(name="yev", bufs=4) as yev_pool,
            tc.tile_pool(name="psA", bufs=4, space="PSUM") as psA,
            tc.tile_pool(name="psY", bufs=4, space="PSUM") as psY,
        ):
            b1t = bias_pool.tile([P, MB1], f32)
            b2t = bias_pool.tile([P, MB2], f32)
            b3t = bias_pool.tile([P, J3], f32)
            nc.sync.dma_start(b1t[:], B1[:])
            nc.sync.dma_start(b2t[:], B2[:])
            nc.sync.dma_start(b3t[:], B3[:])

            c0 = 0
            for ci, T in enumerate(chunks):
                subs = _subs_of(T)
                ns = len(subs)

                xt = xt_pool.tile([P, K1 * max_t], bf16, tag="xt", name=f"xt{ci}")
                for o, ln in subs:
                    for k in range(K1):
                        nc.sync.dma_start(xt[:, k * max_t + o:k * max_t + o + ln],
                                          XT[k, :, c0 + o:c0 + o + ln])
                h1 = h1_pool.tile([P, K2 * max_t], bf16, tag="h1", name=f"h1_{ci}")
                yac = y_pool.tile([P, J3 * max_t], f32, tag="ya", name=f"ya{ci}")

                # ---- layer 1: H1 = gelu(X @ W1 + b1), feature-major ----
                for mb in range(MB1):
                    w1t = w1_pool.tile([P, K1 * P], bf16, tag="w1", name=f"w1_{ci}_{mb}")
                    nc.sync.dma_start(w1t[:], W1A[mb])
                    for si_, (o, ln) in enumerate(subs):
                        ps = psA.tile([P, 512], f32, tag="ps", name=f"l1ps{ci}_{mb}_{si_}")
                        for k in range(K1):
                            nc.tensor.matmul(ps[:, :ln],
                                             w1t[:, k * P:(k + 1) * P],
                                             xt[:, k * max_t + o:k * max_t + o + ln],
                                             start=(k == 0), stop=(k == K1 - 1))
                        nc.scalar.activation(h1[:, mb * max_t + o:mb * max_t + o + ln],
                                             ps[:, :ln], GELU, bias=b1t[:, mb:mb + 1])

                # ---- layer 2 + windowed layer-3 partials ----
                def emit_l3_window(w, h2w, w3w):
                    first = (w == 0)
                    for pair in range(J3 // 2):
                        for jh in range(2):
                            j = 2 * pair + jh
                            pys = {}
                            for si_, (o, ln) in enumerate(subs):
                                py = psY.tile([P, 512], f32, tag="py",
                                              name=f"py{ci}_{w}_{pair}_{jh}_{si_}")
                                pys[si_] = py
                                for wi in range(WIN):
                                    nc.tensor.matmul(
                                        py[:, :ln],
                                        w3w[wi][:, j * P:(j + 1) * P],
                                        h2w[si_][:, wi * 512:wi * 512 + ln],
                                        start=(wi == 0), stop=(wi == WIN - 1))
                            for si_, (o, ln) in enumerate(subs):
                                dst = yac[:, j * max_t + o:j * max_t + o + ln]
                                if first:
                                    nc.vector.tensor_copy(dst, pys[si_][:, :ln])
                                else:
                                    nc.vector.tensor_add(dst, dst, pys[si_][:, :ln])

                pend = None
                for w in range(MB2 // WIN):
                    w3w = {}
                    h2w = {si_: h2_pool.tile([P, WIN * 512], bf16, tag=f"h2_{si_}",
                                             name=f"h2_{ci}_{w}_{si_}")
                           for si_ in range(ns)}
                    for wi in range(WIN):
                        jj = WIN * w + wi
                        w2t = w2_pool.tile([P, K2 * P], bf16, tag="w2",
                                           name=f"w2_{ci}_{jj}")
                        nc.sync.dma_start(w2t[:], W2A[jj])
                        w3t = w3_pool.tile([P, EMB], bf16, tag="w3", name=f"w3_{ci}_{jj}")
                        nc.sync.dma_start(w3t[:], W3A[jj])
                        w3w[wi] = w3t
                        for si_, (o, ln) in enumerate(subs):
                            ps = psA.tile([P, 512], f32, tag="ps",
                                          name=f"l2ps{ci}_{jj}_{si_}")
                            for k in range(K2):
                                nc.tensor.matmul(ps[:, :ln],
                                                 w2t[:, k * P:(k + 1) * P],
                                                 h1[:, k * max_t + o:k * max_t + o + ln],
                                                 start=(k == 0), stop=(k == K2 - 1))
                            nc.scalar.activation(h2w[si_][:, wi * 512:wi * 512 + ln],
                                                 ps[:, :ln], GELU, bias=b2t[:, jj:jj + 1])
                    if pend is not None:
                        emit_l3_window(*pend)
                    pend = (w, h2w, w3w)
                emit_l3_window(*pend)

                # ---- evict Y chunk (bias add) ----
                for j in range(J3):
                    for o, ln in subs:
                        yv = yev_pool.tile([P, 512], f32, tag="yev")
                        nc.scalar.activation(yv[:, :ln],
                                             yac[:, j * max_t + o:j * max_t + o + ln],
                                             IDENT, bias=b3t[:, j:j + 1])
                        nc.sync.dma_start(YT[j, :, c0 + o:c0 + o + ln], yv[:, :ln])
                c0 += T

    _strip_redundant_sem_incs(nc)
    nc.compile()
    return nc


LAST_RUN = {}


def kernel(x, Wg, bg, W1, b1, W2, b2, W3, b3):
    B, N, E = x.shape
    xf = np.ascontiguousarray(x.reshape(-1, E), dtype=np.float32)

    # ---- host gating (float64 ordering is stable vs the fp32 reference) ----
    s = xf.astype(np.float64) @ Wg.astype(np.float64) + bg.astype(np.float64)
    ti = np.argsort(-s, axis=1, kind="stable")[:, :TOPK]
    tv = np.take_along_axis(s, ti, axis=1)
    ex = np.exp(tv - tv.max(axis=1, keepdims=True))
    gates = (ex / ex.sum(axis=1, keepdims=True)).astype(np.float32)

    idx_e, gate_e = [], []
    for e in range(NE):
        m0 = ti[:, 0] == e
        m1 = ti[:, 1] == e
        idx_e.append(np.concatenate([np.nonzero(m0)[0], np.nonzero(m1)[0]]))
        gate_e.append(np.concatenate([gates[m0, 0], gates[m1, 1]]))
    counts = [len(i) for i in idx_e]
    C = max(256, -(-max(counts) // 8) * 8)

    bf = ml_dtypes.bfloat16
    in_maps = []
    for e in range(NE):
        xe = np.zeros((C, EMB), np.float32)
        xe[:counts[e]] = xf[idx_e[e]]
        xt = np.ascontiguousarray(xe.T).reshape(K1, P, C).astype(bf)
        # arena layouts: arena[blk, p, t*128+m] = W[t*128+p, blk*128+m]
        w1a = np.ascontiguousarray(
            W1[e].reshape(K1, P, MB1, P).transpose(2, 1, 0, 3),
            np.float32).reshape(MB1, P, K1 * P).astype(bf)
        w2a = np.ascontiguousarray(
            W2[e].reshape(K2, P, MB2, P).transpose(2, 1, 0, 3),
            np.float32).reshape(MB2, P, K2 * P).astype(bf)
        w3a = np.ascontiguousarray(W3[e], np.float32).reshape(MB2, P, EMB).astype(bf)
        in_maps.append({
            "XT": xt, "W1A": w1a, "W2A": w2a, "W3A": w3a,
            "B1": np.ascontiguousarray(b1[e].reshape(MB1, P).T, np.float32),
            "B2": np.ascontiguousarray(b2[e].reshape(MB2, P).T, np.float32),
            "B3": np.ascontiguousarray(b3[e].reshape(EMB // P, P).T, np.float32),
        })

    trace = bool(int(os.environ.get("KERNEL_TRACE", "0")))
    if trace:
        _install_ntff_hook()
    nc = _build_program(C)
    res = run_bass_kernel_spmd(nc, in_maps, core_ids=list(range(NE)), trace=trace)
    LAST_RUN["exec_time_ns"] = res.exec_time_ns
    LAST_RUN["capacity"] = C

    out = np.zeros_like(xf)
    for e in range(NE):
        yt = res.results[e]["YT"].reshape(EMB, C)
        ye = yt[:, :counts[e]].T
        out[idx_e[e]] += gate_e[e][:, None] * ye
    return out.reshape(B, N, E)


# revision 6
# speedup vs baseline: 1.1945x; 1.1945x over previous
"""MoE (8 experts, top-2) expert-parallel Trainium2 kernel, v2.

Contract: kernel(**inputs) takes the full unsharded inputs and returns the
full [8, 2048, 768] output.  Internally:
  - host computes the gate (scores -> top-2 -> softmax) in float64 and
    dispatches tokens to experts (the "all-to-all" of the sharding hint),
  - each of the 8 NeuronCores runs one expert's 3-layer GELU MLP over its
    routed tokens (padded to a common capacity C) via a Bass/Tile kernel,
  - host combines expert outputs with the gate weights.

v2 changes vs v1:
  - all matmul operands in bf16 (fp32 PSUM accumulation): same PE rate as
    f32r but half the DMA/SBUF traffic and FWL-fast weight loads;
    end-to-end error ~4e-3, well inside the 2e-2 gate.
  - weight "arenas": one contiguous SBUF tile + one DMA per weight block
    (per mb for W1, per jj for W2/W3) instead of many small pool tiles, so
    buffer-reuse WAR waits target only accumulation-group stop matmuls.
  - all token sub-splits are 512 wide (one short tail), minimizing matmul
    instruction count at the PSUM-bank limit.
  - a post-schedule IR pass drops engine-semaphore increments that no wait
    references and renumbers the rest (engines complete in FIFO order, so
    `sem >= v` means "the v-th ticking instruction completed"; unwaited
    ticks are pure overhead, ~15ns each on the PE queue).
"""

import os
import sys
import types

import numpy as np
import ml_dtypes

import concourse.bass as bass  # noqa: F401  (bass must import before mybir use)
import concourse.mybir as mybir
from concourse import bacc
from concourse.tile import TileContext
from concourse.bass_utils import run_bass_kernel_spmd

EMB, HID, HID2 = 768, 3072, 6144
NE, TOPK = 8, 2
P = 128   # partitions
WIN = 4   # layer-2 blocks per layer-3 PSUM accumulation window
K1, K2 = EMB // P, HID // P          # 6, 24 contraction tiles
MB1, MB2 = HID // P, HID2 // P       # 24, 48 output 128-blocks
J3 = EMB // P                        # 6 output blocks of layer 3


def _install_ntff_hook():
    """Make trace=True work when antenv.axon_hooks is missing in the image."""
    try:
        from antenv.axon_hooks import get_axon_ntff_profile_hook  # noqa: F401
        return
    except ImportError:
        pass
    try:
        from trn_agent_boot.trn_boot import _ntff_profile_via_ctypes
        hook = _ntff_profile_via_ctypes('/opt/axon/libaxon_pjrt.so')
        mod = types.ModuleType('antenv.axon_hooks')
        mod.get_axon_ntff_profile_hook = lambda: hook
        sys.modules['antenv.axon_hooks'] = mod
    except Exception:
        pass


# --------------------------------------------------------------------------
# Post-schedule semaphore strip (see module docstring).

def _strip_redundant_sem_incs(nc):
    insts = []
    for f in nc.m.functions:
        for bb in f.blocks:
            for inst in bb.instructions:
                insts.append((bb.name, inst))

    updaters, waiters, blockers = {}, {}, set()
    for bb_name, inst in insts:
        si = inst.sync_info
        if si is None:
            continue
        for u in (si.on_update or []):
            if u.sync_type != "semaphore":
                continue
            if not (u.update_mode == "sem-inc"
                    and (u.update_value in (None, 1))
                    and u.update_reg is None):
                blockers.add(u.id)
            updaters.setdefault(u.id, []).append((bb_name, inst, u))
        for w in (si.on_wait or []):
            if w.sync_type != "semaphore":
                continue
            if w.wait_mode != "sem-ge-imm" or w.wait_reg is not None:
                blockers.add(w.id)
            waiters.setdefault(w.id, []).append((inst, w))

    # Only strip sems whose every updater is a plain compute instruction on
    # one engine FIFO.  DMA instructions can fan out to multiple hardware
    # queues (completion is not FIFO w.r.t. one semaphore), so any sem a DMA
    # updates is left untouched.
    safe_types = ("InstMatmult", "InstActivation", "InstTensorTensor",
                  "InstCopy", "InstTensorReduce", "InstTensorScalarPtr")
    dropped = 0
    for sem_id, ups in updaters.items():
        if sem_id in blockers:
            continue
        if any(type(i).__name__ not in safe_types for _, i, _ in ups):
            continue
        if len({i.engine for _, i, _ in ups}) != 1 or len({b for b, _, _ in ups}) != 1:
            continue
        n = len(ups)
        wts = waiters.get(sem_id, [])
        vals = sorted({w.wait_value for _, w in wts})
        if vals and (vals[0] < 1 or vals[-1] > n):
            continue
        needed = set(vals)
        needed.add(n)  # keep the final tick
        keep = [i + 1 in needed for i in range(n)]
        if all(keep):
            continue
        new_rank, r = {}, 0
        for i in range(n):
            if keep[i]:
                r += 1
            new_rank[i + 1] = r
        for inst, w in wts:
            w.wait_value = new_rank[w.wait_value]
        for i, (_, inst, u) in enumerate(ups):
            if keep[i]:
                continue
            si = inst.sync_info
            inst.sync_info = mybir.SyncInfo(
                on_wait=list(si.on_wait or []),
                on_update=[x for x in si.on_update if x is not u],
            )
            dropped += 1
    return dropped


# --------------------------------------------------------------------------
# Device program.

def _subs_of(t):
    subs, o = [], 0
    while t - o > 512:
        subs.append((o, 512))
        o += 512
    subs.append((o, t - o))
    return subs


def _chunks_of(c):
    chunks = []
    rem = c
    while rem > 1536:
        chunks.append(1024)
        rem -= 1024
    chunks.append(rem)
    return chunks


def _build_program(C):
    f32 = mybir.dt.float32
    bf16 = mybir.dt.bfloat16
    GELU = mybir.ActivationFunctionType.Gelu
    IDENT = mybir.ActivationFunctionType.Identity

    nc = bacc.Bacc(None, target_bir_lowering=False)

    XT = nc.declare_dram_parameter("XT", [K1, P, C], bf16, isOutput=False)
    W1A = nc.declare_dram_parameter("W1A", [MB1, P, K1 * P], bf16, isOutput=False)
    W2A = nc.declare_dram_parameter("W2A", [MB2, P, K2 * P], bf16, isOutput=False)
    W3A = nc.declare_dram_parameter("W3A", [MB2, P, EMB], bf16, isOutput=False)
    B1 = nc.declare_dram_parameter("B1", [P, MB1], f32, isOutput=False)
    B2 = nc.declare_dram_parameter("B2", [P, MB2], f32, isOutput=False)
    B3 = nc.declare_dram_parameter("B3", [P, J3], f32, isOutput=False)
    YT = nc.declare_dram_parameter("YT", [J3, P, C], f32, isOutput=True)

    chunks = _chunks_of(C)
    max_t = max(chunks)

    with TileContext(nc) as tc:
        with (
            tc.tile_pool(name="bias", bufs=1) as bias_pool,
            tc.tile_pool(name="xt", bufs=1) as xt_pool,
            tc.tile_pool(name="h1", bufs=1) as h1_pool,
            tc.tile_pool(name="yac", bufs=1) as y_pool,
            tc.tile_pool(name="w1", bufs=3) as w1_pool,
            tc.tile_pool(name="w2", bufs=3) as w2_pool,
            tc.tile_pool(name="w3", bufs=6) as w3_pool,
            tc.tile_pool(name="h2", bufs=2) as h2_pool,
            tc.tile_pool(name="yev", bufs=4) as yev_pool,
            tc.tile_pool(name="psA", bufs=4, space="PSUM") as psA,
            tc.tile_pool(name="psY", bufs=4, space="PSUM") as psY,
        ):
            b1t = bias_pool.tile([P, MB1], f32)
            b2t = bias_pool.tile([P, MB2], f32)
            b3t = bias_pool.tile([P, J3], f32)
            nc.sync.dma_start(b1t[:], B1[:])
            nc.sync.dma_start(b2t[:], B2[:])
            nc.sync.dma_start(b3t[:], B3[:])

            c0 = 0
            for ci, T in enumerate(chunks):
                subs = _subs_of(T)
                ns = len(subs)

                xt = xt_pool.tile([P, K1 * max_t], bf16, tag="xt", name=f"xt{ci}")
                for o, ln in subs:
                    for k in range(K1):
                        nc.sync.dma_start(xt[:, k * max_t + o:k * max_t + o + ln],
                                          XT[k, :, c0 + o:c0 + o + ln])
                h1 = h1_pool.tile([P, K2 * max_t], bf16, tag="h1", name=f"h1_{ci}")
                yac = y_pool.tile([P, J3 * max_t], f32, tag="ya", name=f"ya{ci}")

                # ---- layer 1: H1 = gelu(X @ W1 + b1), feature-major ----
                # k-outer / sub-inner so consecutive matmuls alternate PSUM
                # banks (same-bank back-to-back serializes drain vs fill).
                for mb in range(MB1):
                    w1t = w1_pool.tile([P, K1 * P], bf16, tag="w1", name=f"w1_{ci}_{mb}")
                    nc.sync.dma_start(w1t[:], W1A[mb])
                    ps = {si_: psA.tile([P, 512], f32, tag="ps",
                                        name=f"l1ps{ci}_{mb}_{si_}")
                          for si_ in range(ns)}
                    for k in range(K1):
                        for si_, (o, ln) in enumerate(subs):
                            nc.tensor.matmul(ps[si_][:, :ln],
                                             w1t[:, k * P:(k + 1) * P],
                                             xt[:, k * max_t + o:k * max_t + o + ln],
                                             start=(k == 0), stop=(k == K1 - 1))
                    for si_, (o, ln) in enumerate(subs):
                        nc.scalar.activation(h1[:, mb * max_t + o:mb * max_t + o + ln],
                                             ps[si_][:, :ln], GELU, bias=b1t[:, mb:mb + 1])

                # ---- layer 2 + windowed layer-3 partials ----
                def emit_l3_window(w, h2w, w3w):
                    first = (w == 0)
                    for pair in range(J3 // 2):
                        for jh in range(2):
                            j = 2 * pair + jh
                            pys = {si_: psY.tile([P, 512], f32, tag="py",
                                                 name=f"py{ci}_{w}_{pair}_{jh}_{si_}")
                                   for si_ in range(ns)}
                            for wi in range(WIN):
                                for si_, (o, ln) in enumerate(subs):
                                    nc.tensor.matmul(
                                        pys[si_][:, :ln],
                                        w3w[wi][:, j * P:(j + 1) * P],
                                        h2w[si_][:, wi * 512:wi * 512 + ln],
                                        start=(wi == 0), stop=(wi == WIN - 1))
                            for si_, (o, ln) in enumerate(subs):
                                dst = yac[:, j * max_t + o:j * max_t + o + ln]
                                if first:
                                    nc.vector.tensor_copy(dst, pys[si_][:, :ln])
                                else:
                                    nc.vector.tensor_add(dst, dst, pys[si_][:, :ln])

                pend = None
                for w in range(MB2 // WIN):
                    w3w = {}
                    h2w = {si_: h2_pool.tile([P, WIN * 512], bf16, tag=f"h2_{si_}",
                                             name=f"h2_{ci}_{w}_{si_}")
                           for si_ in range(ns)}
                    for wi in range(WIN):
                        jj = WIN * w + wi
                        w2t = w2_pool.tile([P, K2 * P], bf16, tag="w2",
                                           name=f"w2_{ci}_{jj}")
                        nc.sync.dma_start(w2t[:], W2A[jj])
                        w3t = w3_pool.tile([P, EMB], bf16, tag="w3", name=f"w3_{ci}_{jj}")
                        nc.sync.dma_start(w3t[:], W3A[jj])
                        w3w[wi] = w3t
                        ps = {si_: psA.tile([P, 512], f32, tag="ps",
                                            name=f"l2ps{ci}_{jj}_{si_}")
                              for si_ in range(ns)}
                        for k in range(K2):
                            for si_, (o, ln) in enumerate(subs):
                                nc.tensor.matmul(ps[si_][:, :ln],
                                                 w2t[:, k * P:(k + 1) * P],
                                                 h1[:, k * max_t + o:k * max_t + o + ln],
                                                 start=(k == 0), stop=(k == K2 - 1))
                        for si_, (o, ln) in enumerate(subs):
                            nc.scalar.activation(h2w[si_][:, wi * 512:wi * 512 + ln],
                                                 ps[si_][:, :ln], GELU, bias=b2t[:, jj:jj + 1])
                    if pend is not None:
                        emit_l3_window(*pend)
                    pend = (w, h2w, w3w)
                emit_l3_window(*pend)

                # ---- evict Y chunk (bias add) ----
                for j in range(J3):
                    for o, ln in subs:
                        yv = yev_pool.tile([P, 512], f32, tag="yev")
                        nc.scalar.activation(yv[:, :ln],
                                             yac[:, j * max_t + o:j * max_t + o + ln],
                                             IDENT, bias=b3t[:, j:j + 1])
                        nc.sync.dma_start(YT[j, :, c0 + o:c0 + o + ln], yv[:, :ln])
                c0 += T

    _strip_redundant_sem_incs(nc)
    nc.compile()
    return nc


LAST_RUN = {}


def kernel(x, Wg, bg, W1, b1, W2, b2, W3, b3):
    B, N, E = x.shape
    xf = np.ascontiguousarray(x.reshape(-1, E), dtype=np.float32)

    # ---- host gating (float64 ordering is stable vs the fp32 reference) ----
    s = xf.astype(np.float64) @ Wg.astype(np.float64) + bg.astype(np.float64)
    ti = np.argsort(-s, axis=1, kind="stable")[:, :TOPK]
    tv = np.take_along_axis(s, ti, axis=1)
    ex = np.exp(tv - tv.max(axis=1, keepdims=True))
    gates = (ex / ex.sum(axis=1, keepdims=True)).astype(np.float32)

    idx_e, gate_e = [], []
    for e in range(NE):
        m0 = ti[:, 0] == e
        m1 = ti[:, 1] == e
        idx_e.append(np.concatenate([np.nonzero(m0)[0], np.nonzero(m1)[0]]))
        gate_e.append(np.concatenate([gates[m0, 0], gates[m1, 1]]))
    counts = [len(i) for i in idx_e]
    C = max(256, -(-max(counts) // 8) * 8)

    bf = ml_dtypes.bfloat16
    in_maps = []
    for e in range(NE):
        xe = np.zeros((C, EMB), np.float32)
        xe[:counts[e]] = xf[idx_e[e]]
        xt = np.ascontiguousarray(xe.T).reshape(K1, P, C).astype(bf)
        # arena layouts: arena[blk, p, t*128+m] = W[t*128+p, blk*128+m]
        w1a = np.ascontiguousarray(
            W1[e].reshape(K1, P, MB1, P).transpose(2, 1, 0, 3),
            np.float32).reshape(MB1, P, K1 * P).astype(bf)
        w2a = np.ascontiguousarray(
            W2[e].reshape(K2, P, MB2, P).transpose(2, 1, 0, 3),
            np.float32).reshape(MB2, P, K2 * P).astype(bf)
        w3a = np.ascontiguousarray(W3[e], np.float32).reshape(MB2, P, EMB).astype(bf)
        in_maps.append({
            "XT": xt, "W1A": w1a, "W2A": w2a, "W3A": w3a,
            "B1": np.ascontiguousarray(b1[e].reshape(MB1, P).T, np.float32),
            "B2": np.ascontiguousarray(b2[e].reshape(MB2, P).T, np.float32),
            "B3": np.ascontiguousarray(b3[e].reshape(EMB // P, P).T, np.float32),
        })

    trace = bool(int(os.environ.get("KERNEL_TRACE", "0")))
    if trace:
        _install_ntff_hook()
    nc = _build_program(C)
    res = run_bass_kernel_spmd(nc, in_maps, core_ids=list(range(NE)), trace=trace)
    LAST_RUN["exec_time_ns"] = res.exec_time_ns
    LAST_RUN["capacity"] = C

    out = np.zeros_like(xf)
    for e in range(NE):
        yt = res.results[e]["YT"].reshape(EMB, C)
        ye = yt[:, :counts[e]].T
        out[idx_e[e]] += gate_e[e][:, None] * ye
    return out.reshape(B, N, E)


# revision 11
# speedup vs baseline: 1.1978x; 1.0028x over previous
"""MoE (8 experts, top-2) expert-parallel Trainium2 kernel, v2.

Contract: kernel(**inputs) takes the full unsharded inputs and returns the
full [8, 2048, 768] output.  Internally:
  - host computes the gate (scores -> top-2 -> softmax) in float64 and
    dispatches tokens to experts (the "all-to-all" of the sharding hint),
  - each of the 8 NeuronCores runs one expert's 3-layer GELU MLP over its
    routed tokens (padded to a common capacity C) via a Bass/Tile kernel,
  - host combines expert outputs with the gate weights.

v2 changes vs v1:
  - all matmul operands in bf16 (fp32 PSUM accumulation): same PE rate as
    f32r but half the DMA/SBUF traffic and FWL-fast weight loads;
    end-to-end error ~4e-3, well inside the 2e-2 gate.
  - weight "arenas": one contiguous SBUF tile + one DMA per weight block
    (per mb for W1, per jj for W2/W3) instead of many small pool tiles, so
    buffer-reuse WAR waits target only accumulation-group stop matmuls.
  - all token sub-splits are 512 wide (one short tail), minimizing matmul
    instruction count at the PSUM-bank limit.
  - a post-schedule IR pass drops engine-semaphore increments that no wait
    references and renumbers the rest (engines complete in FIFO order, so
    `sem >= v` means "the v-th ticking instruction completed"; unwaited
    ticks are pure overhead, ~15ns each on the PE queue).
"""

import os
import sys
import types

import numpy as np
import ml_dtypes

import concourse.bass as bass  # noqa: F401  (bass must import before mybir use)
import concourse.mybir as mybir
from concourse import bacc
from concourse.tile import TileContext
from concourse.bass_utils import run_bass_kernel_spmd

EMB, HID, HID2 = 768, 3072, 6144
NE, TOPK = 8, 2
P = 128   # partitions
WIN = 4   # layer-2 blocks per layer-3 PSUM accumulation window
K1, K2 = EMB // P, HID // P          # 6, 24 contraction tiles
MB1, MB2 = HID // P, HID2 // P       # 24, 48 output 128-blocks
J3 = EMB // P                        # 6 output blocks of layer 3


def _install_ntff_hook():
    """Make trace=True work when antenv.axon_hooks is missing in the image."""
    try:
        from antenv.axon_hooks import get_axon_ntff_profile_hook  # noqa: F401
        return
    except ImportError:
        pass
    try:
        from trn_agent_boot.trn_boot import _ntff_profile_via_ctypes
        hook = _ntff_profile_via_ctypes('/opt/axon/libaxon_pjrt.so')
        mod = types.ModuleType('antenv.axon_hooks')
        mod.get_axon_ntff_profile_hook = lambda: hook
        sys.modules['antenv.axon_hooks'] = mod
    except Exception:
        pass


# --------------------------------------------------------------------------
# Post-schedule semaphore strip (see module docstring).

def _strip_redundant_sem_incs(nc):
    insts = []
    for f in nc.m.functions:
        for bb in f.blocks:
            for inst in bb.instructions:
                insts.append((bb.name, inst))

    updaters, waiters, blockers = {}, {}, set()
    for bb_name, inst in insts:
        si = inst.sync_info
        if si is None:
            continue
        for u in (si.on_update or []):
            if u.sync_type != "semaphore":
                continue
            if not (u.update_mode == "sem-inc"
                    and (u.update_value in (None, 1))
                    and u.update_reg is None):
                blockers.add(u.id)
            updaters.setdefault(u.id, []).append((bb_name, inst, u))
        for w in (si.on_wait or []):
            if w.sync_type != "semaphore":
                continue
            if w.wait_mode != "sem-ge-imm" or w.wait_reg is not None:
                blockers.add(w.id)
            waiters.setdefault(w.id, []).append((inst, w))

    # Only strip sems whose every updater is a plain compute instruction on
    # one engine FIFO.  DMA instructions can fan out to multiple hardware
    # queues (completion is not FIFO w.r.t. one semaphore), so any sem a DMA
    # updates is left untouched.
    safe_types = ("InstMatmult", "InstActivation", "InstTensorTensor",
                  "InstCopy", "InstTensorReduce", "InstTensorScalarPtr")
    dropped = 0
    for sem_id, ups in updaters.items():
        if sem_id in blockers:
            continue
        if any(type(i).__name__ not in safe_types for _, i, _ in ups):
            continue
        if len({i.engine for _, i, _ in ups}) != 1 or len({b for b, _, _ in ups}) != 1:
            continue
        n = len(ups)
        wts = waiters.get(sem_id, [])
        vals = sorted({w.wait_value for _, w in wts})
        if vals and (vals[0] < 1 or vals[-1] > n):
            continue
        needed = set(vals)
        needed.add(n)  # keep the final tick
        keep = [i + 1 in needed for i in range(n)]
        if all(keep):
            continue
        new_rank, r = {}, 0
        for i in range(n):
            if keep[i]:
                r += 1
            new_rank[i + 1] = r
        for inst, w in wts:
            w.wait_value = new_rank[w.wait_value]
        for i, (_, inst, u) in enumerate(ups):
            if keep[i]:
                continue
            si = inst.sync_info
            inst.sync_info = mybir.SyncInfo(
                on_wait=list(si.on_wait or []),
                on_update=[x for x in si.on_update if x is not u],
            )
            dropped += 1
    return dropped


# --------------------------------------------------------------------------
# Device program.

def _subs_of(t):
    subs, o = [], 0
    while t - o > 512:
        subs.append((o, 512))
        o += 512
    subs.append((o, t - o))
    return subs


def _chunks_of(c):
    chunks = []
    rem = c
    while rem > 1536:
        chunks.append(1024)
        rem -= 1024
    chunks.append(rem)
    return chunks


def _build_program(C):
    f32 = mybir.dt.float32
    bf16 = mybir.dt.bfloat16
    GELU = mybir.ActivationFunctionType.Gelu
    IDENT = mybir.ActivationFunctionType.Identity

    nc = bacc.Bacc(None, target_bir_lowering=False)

    XT = nc.declare_dram_parameter("XT", [K1, P, C], bf16, isOutput=False)
    W1A = nc.declare_dram_parameter("W1A", [MB1, P, K1 * P], bf16, isOutput=False)
    W2A = nc.declare_dram_parameter("W2A", [MB2, P, K2 * P], bf16, isOutput=False)
    W3A = nc.declare_dram_parameter("W3A", [MB2, P, EMB], bf16, isOutput=False)
    B1 = nc.declare_dram_parameter("B1", [P, MB1], f32, isOutput=False)
    B2 = nc.declare_dram_parameter("B2", [P, MB2], f32, isOutput=False)
    B3 = nc.declare_dram_parameter("B3", [P, J3], f32, isOutput=False)
    YT = nc.declare_dram_parameter("YT", [J3, P, C], f32, isOutput=True)

    chunks = _chunks_of(C)
    max_t = max(chunks)

    with TileContext(nc) as tc:
        with (
            tc.tile_pool(name="bias", bufs=1) as bias_pool,
            tc.tile_pool(name="xt", bufs=2) as xt_pool,
            tc.tile_pool(name="h1", bufs=1) as h1_pool,
            tc.tile_pool(name="yac", bufs=1) as y_pool,
            tc.tile_pool(name="w1", bufs=3) as w1_pool,
            tc.tile_pool(name="w2", bufs=3) as w2_pool,
            tc.tile_pool(name="w3", bufs=6) as w3_pool,
            tc.tile_pool(name="h2", bufs=2) as h2_pool,
            tc.tile_pool(name="yev", bufs=4) as yev_pool,
            tc.tile_pool(name="psA", bufs=4, space="PSUM") as psA,
            tc.tile_pool(name="psY", bufs=4, space="PSUM") as psY,
        ):
            b1t = bias_pool.tile([P, MB1], f32)
            b2t = bias_pool.tile([P, MB2], f32)
            b3t = bias_pool.tile([P, J3], f32)
            nc.sync.dma_start(b1t[:], B1[:])
            nc.sync.dma_start(b2t[:], B2[:])
            nc.sync.dma_start(b3t[:], B3[:])

            c0 = 0
            for ci, T in enumerate(chunks):
                subs = _subs_of(T)
                ns = len(subs)

                # prefetch the first weight blocks before the bulk X DMAs so
                # the PE's first accumulation group starts as early as possible
                w1_pre = {}
                if ci == 0:
                    for mb in range(2):
                        w1t = w1_pool.tile([P, K1 * P], bf16, tag="w1",
                                           name=f"w1_{ci}_{mb}")
                        nc.sync.dma_start(w1t[:], W1A[mb])
                        w1_pre[mb] = w1t

                xt = xt_pool.tile([P, K1 * max_t], bf16, tag="xt", name=f"xt{ci}")
                for o, ln in subs:
                    for k in range(K1):
                        nc.sync.dma_start(xt[:, k * max_t + o:k * max_t + o + ln],
                                          XT[k, :, c0 + o:c0 + o + ln])
                h1 = h1_pool.tile([P, K2 * max_t], bf16, tag="h1", name=f"h1_{ci}")
                yac = y_pool.tile([P, J3 * max_t], f32, tag="ya", name=f"ya{ci}")

                # ---- layer 1: H1 = gelu(X @ W1 + b1), feature-major ----
                # k-outer / sub-inner so consecutive matmuls alternate PSUM
                # banks (same-bank back-to-back serializes drain vs fill).
                for mb in range(MB1):
                    if mb in w1_pre:
                        w1t = w1_pre[mb]
                    else:
                        w1t = w1_pool.tile([P, K1 * P], bf16, tag="w1",
                                           name=f"w1_{ci}_{mb}")
                        nc.sync.dma_start(w1t[:], W1A[mb])
                    ps = {si_: psA.tile([P, 512], f32, tag="ps",
                                        name=f"l1ps{ci}_{mb}_{si_}")
                          for si_ in range(ns)}
                    for k in range(K1):
                        for si_, (o, ln) in enumerate(subs):
                            nc.tensor.matmul(ps[si_][:, :ln],
                                             w1t[:, k * P:(k + 1) * P],
                                             xt[:, k * max_t + o:k * max_t + o + ln],
                                             start=(k == 0), stop=(k == K1 - 1))
                    for si_, (o, ln) in enumerate(subs):
                        nc.scalar.activation(h1[:, mb * max_t + o:mb * max_t + o + ln],
                                             ps[si_][:, :ln], GELU, bias=b1t[:, mb:mb + 1])

                # ---- layer 2 + windowed layer-3 partials ----
                def emit_l3_window(w, h2w, w3w, last=False):
                    first = (w == 0)
                    for pair in range(J3 // 2):
                        for jh in range(2):
                            j = 2 * pair + jh
                            pys = {si_: psY.tile([P, 512], f32, tag="py",
                                                 name=f"py{ci}_{w}_{pair}_{jh}_{si_}")
                                   for si_ in range(ns)}
                            for wi in range(WIN):
                                for si_, (o, ln) in enumerate(subs):
                                    nc.tensor.matmul(
                                        pys[si_][:, :ln],
                                        w3w[wi][:, j * P:(j + 1) * P],
                                        h2w[si_][:, wi * 512:wi * 512 + ln],
                                        start=(wi == 0), stop=(wi == WIN - 1))
                            for si_, (o, ln) in enumerate(subs):
                                dst = yac[:, j * max_t + o:j * max_t + o + ln]
                                if first:
                                    nc.vector.tensor_copy(dst, pys[si_][:, :ln])
                                else:
                                    nc.vector.tensor_add(dst, dst, pys[si_][:, :ln])
                            if last:
                                # evict this j immediately; overlaps the
                                # remaining pairs' matmuls
                                for o, ln in subs:
                                    yv = yev_pool.tile([P, 512], f32, tag="yev")
                                    nc.scalar.activation(
                                        yv[:, :ln],
                                        yac[:, j * max_t + o:j * max_t + o + ln],
                                        IDENT, bias=b3t[:, j:j + 1])
                                    nc.sync.dma_start(
                                        YT[j, :, c0 + o:c0 + o + ln], yv[:, :ln])

                pend = None
                for w in range(MB2 // WIN):
                    w3w = {}
                    h2w = {si_: h2_pool.tile([P, WIN * 512], bf16, tag=f"h2_{si_}",
                                             name=f"h2_{ci}_{w}_{si_}")
                           for si_ in range(ns)}
                    for wi in range(WIN):
                        jj = WIN * w + wi
                        w2t = w2_pool.tile([P, K2 * P], bf16, tag="w2",
                                           name=f"w2_{ci}_{jj}")
                        nc.sync.dma_start(w2t[:], W2A[jj])
                        w3t = w3_pool.tile([P, EMB], bf16, tag="w3", name=f"w3_{ci}_{jj}")
                        nc.sync.dma_start(w3t[:], W3A[jj])
                        w3w[wi] = w3t
                        ps = {si_: psA.tile([P, 512], f32, tag="ps",
                                            name=f"l2ps{ci}_{jj}_{si_}")
                              for si_ in range(ns)}
                        for k in range(K2):
                            for si_, (o, ln) in enumerate(subs):
                                nc.tensor.matmul(ps[si_][:, :ln],
                                                 w2t[:, k * P:(k + 1) * P],
                                                 h1[:, k * max_t + o:k * max_t + o + ln],
                                                 start=(k == 0), stop=(k == K2 - 1))
                        for si_, (o, ln) in enumerate(subs):
                            nc.scalar.activation(h2w[si_][:, wi * 512:wi * 512 + ln],
                                                 ps[si_][:, :ln], GELU, bias=b2t[:, jj:jj + 1])
                    if pend is not None:
                        emit_l3_window(*pend)
                    pend = (w, h2w, w3w)
                emit_l3_window(*pend, last=True)
                c0 += T

    _strip_redundant_sem_incs(nc)
    nc.compile()
    return nc


LAST_RUN = {}


def kernel(x, Wg, bg, W1, b1, W2, b2, W3, b3):
    B, N, E = x.shape
    xf = np.ascontiguousarray(x.reshape(-1, E), dtype=np.float32)

    # ---- host gating (float64 ordering is stable vs the fp32 reference) ----
    s = xf.astype(np.float64) @ Wg.astype(np.float64) + bg.astype(np.float64)
    ti = np.argsort(-s, axis=1, kind="stable")[:, :TOPK]
    tv = np.take_along_axis(s, ti, axis=1)
    ex = np.exp(tv - tv.max(axis=1, keepdims=True))
    gates = (ex / ex.sum(axis=1, keepdims=True)).astype(np.float32)

    idx_e, gate_e = [], []
    for e in range(NE):
        m0 = ti[:, 0] == e
        m1 = ti[:, 1] == e
        idx_e.append(np.concatenate([np.nonzero(m0)[0], np.nonzero(m1)[0]]))
        gate_e.append(np.concatenate([gates[m0, 0], gates[m1, 1]]))
    counts = [len(i) for i in idx_e]
    C = max(256, -(-max(counts) // 8) * 8)

    bf = ml_dtypes.bfloat16
    in_maps = []
    for e in range(NE):
        xe = np.zeros((C, EMB), np.float32)
        xe[:counts[e]] = xf[idx_e[e]]
        xt = np.ascontiguousarray(xe.T).reshape(K1, P, C).astype(bf)
        # arena layouts: arena[blk, p, t*128+m] = W[t*128+p, blk*128+m]
        w1a = np.ascontiguousarray(
            W1[e].reshape(K1, P, MB1, P).transpose(2, 1, 0, 3),
            np.float32).reshape(MB1, P, K1 * P).astype(bf)
        w2a = np.ascontiguousarray(
            W2[e].reshape(K2, P, MB2, P).transpose(2, 1, 0, 3),
            np.float32).reshape(MB2, P, K2 * P).astype(bf)
        w3a = np.ascontiguousarray(W3[e], np.float32).reshape(MB2, P, EMB).astype(bf)
        in_maps.append({
            "XT": xt, "W1A": w1a, "W2A": w2a, "W3A": w3a,
            "B1": np.ascontiguousarray(b1[e].reshape(MB1, P).T, np.float32),
            "B2": np.ascontiguousarray(b2[e].reshape(MB2, P).T, np.float32),
            "B3": np.ascontiguousarray(b3[e].reshape(EMB // P, P).T, np.float32),
        })

    trace = bool(int(os.environ.get("KERNEL_TRACE", "0")))
    if trace:
        _install_ntff_hook()
    nc = _build_program(C)
    res = run_bass_kernel_spmd(nc, in_maps, core_ids=list(range(NE)), trace=trace)
    LAST_RUN["exec_time_ns"] = res.exec_time_ns
    LAST_RUN["capacity"] = C

    out = np.zeros_like(xf)
    for e in range(NE):
        yt = res.results[e]["YT"].reshape(EMB, C)
        ye = yt[:, :counts[e]].T
        out[idx_e[e]] += gate_e[e][:, None] * ye
    return out.reshape(B, N, E)


# revision 15
# speedup vs baseline: 1.1994x; 1.0013x over previous
"""MoE (8 experts, top-2) expert-parallel Trainium2 kernel, v3.

Contract: kernel(**inputs) takes the full unsharded inputs and returns the
full [8, 2048, 768] output.  Internally:
  - host computes the gate (scores -> top-2 -> softmax) in float64 and
    dispatches tokens to experts (the "all-to-all" of the sharding hint),
  - each of the 8 NeuronCores runs a 3-layer GELU MLP over routed tokens
    via a Bass/Tile kernel,
  - host combines expert outputs with the gate weights.

Performance structure (see v2 notes):
  - all matmul operands bf16 (fp32 PSUM accumulation): same PE rate as f32r,
    half the DMA/SBUF, FWL-fast weight loads; error ~4e-3 vs 2e-2 gate.
  - weight "arenas": one contiguous SBUF tile + one DMA per weight block.
  - k-outer/sub-inner matmul interleave so consecutive matmuls alternate
    PSUM banks (same-bank back-to-back serializes drain vs fill, +45ns/MM).
  - post-schedule IR pass drops engine-semaphore increments no wait
    references (engines are FIFO; unwaited ticks cost ~15ns each on PE).
  - load balance: instead of padding every core to the max expert load,
    each core runs C0 tokens of its own expert plus two small overflow
    slots (V1, V2) that can host any expert's tokens with their own weight
    inputs; a host-side packer spreads overloaded experts' tails across
    underloaded cores.  Capacity drops from max(count) to ~avg(count).
"""

import os
import sys
import types

import numpy as np
import ml_dtypes

import concourse.bass as bass  # noqa: F401  (bass must import before mybir use)
import concourse.mybir as mybir
from concourse import bacc
from concourse.tile import TileContext
from concourse.bass_utils import run_bass_kernel_spmd

EMB, HID, HID2 = 768, 3072, 6144
NE, TOPK = 8, 2
P = 128   # partitions
WIN = 4   # layer-2 blocks per layer-3 PSUM accumulation window
K1, K2 = EMB // P, HID // P          # 6, 24 contraction tiles
MB1, MB2 = HID // P, HID2 // P       # 24, 48 output 128-blocks
J3 = EMB // P                        # 6 output blocks of layer 3


def _install_ntff_hook():
    """Make trace=True work when antenv.axon_hooks is missing in the image."""
    try:
        from antenv.axon_hooks import get_axon_ntff_profile_hook  # noqa: F401
        return
    except ImportError:
        pass
    try:
        from trn_agent_boot.trn_boot import _ntff_profile_via_ctypes
        hook = _ntff_profile_via_ctypes('/opt/axon/libaxon_pjrt.so')
        mod = types.ModuleType('antenv.axon_hooks')
        mod.get_axon_ntff_profile_hook = lambda: hook
        sys.modules['antenv.axon_hooks'] = mod
    except Exception:
        pass


# --------------------------------------------------------------------------
# Post-schedule semaphore strip.
#
# Engines complete instructions in FIFO order, so a wait `sem >= v` means
# "the v-th ticking instruction on that engine completed".  Increments of
# instructions whose tick value no wait references are pure dispatch
# overhead (~15ns each on the PE queue); drop them and renumber the rest.
# Semaphores updated by DMA instructions are left untouched: one DMA can
# fan out to several hardware queues, so its completions are not FIFO
# w.r.t. a single semaphore.

def _strip_redundant_sem_incs(nc):
    insts = []
    for f in nc.m.functions:
        for bb in f.blocks:
            for inst in bb.instructions:
                insts.append((bb.name, inst))

    updaters, waiters, blockers = {}, {}, set()
    for bb_name, inst in insts:
        si = inst.sync_info
        if si is None:
            continue
        for u in (si.on_update or []):
            if u.sync_type != "semaphore":
                continue
            if not (u.update_mode == "sem-inc"
                    and (u.update_value in (None, 1))
                    and u.update_reg is None):
                blockers.add(u.id)
            updaters.setdefault(u.id, []).append((bb_name, inst, u))
        for w in (si.on_wait or []):
            if w.sync_type != "semaphore":
                continue
            if w.wait_mode != "sem-ge-imm" or w.wait_reg is not None:
                blockers.add(w.id)
            waiters.setdefault(w.id, []).append((inst, w))

    safe_types = ("InstMatmult", "InstActivation", "InstTensorTensor",
                  "InstCopy", "InstTensorReduce", "InstTensorScalarPtr")
    dropped = 0
    for sem_id, ups in updaters.items():
        if sem_id in blockers:
            continue
        if any(type(i).__name__ not in safe_types for _, i, _ in ups):
            continue
        if len({i.engine for _, i, _ in ups}) != 1 or len({b for b, _, _ in ups}) != 1:
            continue
        n = len(ups)
        wts = waiters.get(sem_id, [])
        vals = sorted({w.wait_value for _, w in wts})
        if vals and (vals[0] < 1 or vals[-1] > n):
            continue
        needed = set(vals)
        needed.add(n)  # keep the final tick
        keep = [i + 1 in needed for i in range(n)]
        if all(keep):
            continue
        new_rank, r = {}, 0
        for i in range(n):
            if keep[i]:
                r += 1
            new_rank[i + 1] = r
        for inst, w in wts:
            w.wait_value = new_rank[w.wait_value]
        for i, (_, inst, u) in enumerate(ups):
            if keep[i]:
                continue
            si = inst.sync_info
            inst.sync_info = mybir.SyncInfo(
                on_wait=list(si.on_wait or []),
                on_update=[x for x in si.on_update if x is not u],
            )
            dropped += 1
    return dropped


# --------------------------------------------------------------------------
# Capacity planning (host side).

def _subs_of(t):
    subs, o = [], 0
    while t - o > 512:
        subs.append((o, 512))
        o += 512
    subs.append((o, t - o))
    return subs


def _chunks_of(c):
    chunks = []
    rem = c
    while rem > 1536:
        chunks.append(1024)
        rem -= 1024
    chunks.append(rem)
    # largest chunk first: the tail chunk drains the shortest pipeline
    return sorted(chunks, reverse=True)


def _pack_overflow(overflows, V1, V2):
    """Pack per-expert overflow amounts into 8 bins of V1 plus 8 bins of V2
    (each bin single-expert; an expert may span several bins).  Returns
    per-expert (a, b) bin counts or None."""
    order = sorted(range(len(overflows)), key=lambda i: -overflows[i])
    items = [overflows[i] for i in order]

    def combos(o):
        out = []
        for a in range(0, 9):
            rem = o - a * V1
            b = 0 if rem <= 0 else -(-rem // V2)
            if b > 8:
                continue
            out.append((a, b))
        out = [c for c in out
               if not any(d != c and d[0] <= c[0] and d[1] <= c[1] for d in out)]
        return out

    def dfs(i, r1, r2):
        if i == len(items):
            return []
        for a, b in combos(items[i]):
            if a <= r1 and b <= r2:
                rest = dfs(i + 1, r1 - a, r2 - b)
                if rest is not None:
                    return [(a, b)] + rest
        return None

    sol = dfs(0, 8, 8)
    if sol is None:
        return None
    res = [None] * len(overflows)
    for pos, i in enumerate(order):
        res[i] = sol[pos]
    return res


def _search_capacity(counts):
    """Minimize C0+V1+V2 such that every expert fits in its main slot of C0
    plus overflow bins.  Returns (C0, V1, V2, per-expert bins) or None."""
    c_triv = max(256, -(-max(counts) // 8) * 8)
    total = sum(counts)
    for cap in range(-(-total // 64) * 8, c_triv, 8):
        for V2 in range(32, 257, 8):
            for V1 in range(V2, 385, 8):
                C0 = cap - V1 - V2
                if C0 < 2048:
                    continue
                ov = [max(0, c - C0) for c in counts]
                if sum(ov) > 8 * (V1 + V2):
                    continue
                sol = _pack_overflow(ov, V1, V2)
                if sol is not None:
                    return C0, V1, V2, sol
    return None


# --------------------------------------------------------------------------
# Device program.

def _build_program(C0, V1, V2):
    f32 = mybir.dt.float32
    bf16 = mybir.dt.bfloat16
    GELU = mybir.ActivationFunctionType.Gelu
    IDENT = mybir.ActivationFunctionType.Identity

    C = C0 + V1 + V2
    nc = bacc.Bacc(None, target_bir_lowering=False)

    XT = nc.declare_dram_parameter("XT", [K1, P, C], bf16, isOutput=False)
    YT = nc.declare_dram_parameter("YT", [J3, P, C], f32, isOutput=True)

    classes = ["M"] + (["V1"] if V1 else []) + (["V2"] if V2 else [])
    par = {}
    for cls in classes:
        sfx = "" if cls == "M" else cls
        par[cls] = dict(
            W1=nc.declare_dram_parameter(f"W1A{sfx}", [MB1, P, K1 * P], bf16,
                                         isOutput=False),
            W2=nc.declare_dram_parameter(f"W2A{sfx}", [MB2, P, K2 * P], bf16,
                                         isOutput=False),
            W3=nc.declare_dram_parameter(f"W3A{sfx}", [MB2, P, EMB], bf16,
                                         isOutput=False),
            B1=nc.declare_dram_parameter(f"B1{sfx}", [P, MB1], f32, isOutput=False),
            B2=nc.declare_dram_parameter(f"B2{sfx}", [P, MB2], f32, isOutput=False),
            B3=nc.declare_dram_parameter(f"B3{sfx}", [P, J3], f32, isOutput=False),
        )

    chunk_list = [(T, "M") for T in _chunks_of(C0)]
    if V1:
        chunk_list.append((V1, "V1"))
    if V2:
        chunk_list.append((V2, "V2"))
    max_t = max(T for T, _ in chunk_list)

    with TileContext(nc) as tc:
        with (
            tc.tile_pool(name="bias", bufs=1) as bias_pool,
            tc.tile_pool(name="xt", bufs=2) as xt_pool,
            tc.tile_pool(name="h1", bufs=1) as h1_pool,
            tc.tile_pool(name="yac", bufs=1) as y_pool,
            tc.tile_pool(name="w1", bufs=3) as w1_pool,
            tc.tile_pool(name="w2", bufs=3) as w2_pool,
            tc.tile_pool(name="w3", bufs=6) as w3_pool,
            tc.tile_pool(name="h2", bufs=2) as h2_pool,
            tc.tile_pool(name="yev", bufs=4) as yev_pool,
            tc.tile_pool(name="psA", bufs=4, space="PSUM") as psA,
            tc.tile_pool(name="psY", bufs=4, space="PSUM") as psY,
        ):
            bias_t = {}
            for cls in classes:
                b1t = bias_pool.tile([P, MB1], f32, name=f"b1{cls}")
                b2t = bias_pool.tile([P, MB2], f32, name=f"b2{cls}")
                b3t = bias_pool.tile([P, J3], f32, name=f"b3{cls}")
                nc.sync.dma_start(b1t[:], par[cls]["B1"][:])
                nc.sync.dma_start(b2t[:], par[cls]["B2"][:])
                nc.sync.dma_start(b3t[:], par[cls]["B3"][:])
                bias_t[cls] = (b1t, b2t, b3t)

            c0 = 0
            for ci, (T, cls) in enumerate(chunk_list):
                subs = _subs_of(T)
                ns = len(subs)
                W1P, W2P, W3P = par[cls]["W1"], par[cls]["W2"], par[cls]["W3"]
                b1t, b2t, b3t = bias_t[cls]

                # prefetch the first weight blocks before the bulk X DMAs so
                # the PE's first accumulation group starts as early as possible
                w1_pre = {}
                if ci == 0:
                    for mb in range(2):
                        w1t = w1_pool.tile([P, K1 * P], bf16, tag="w1",
                                           name=f"w1_{ci}_{mb}")
                        nc.sync.dma_start(w1t[:], W1P[mb])
                        w1_pre[mb] = w1t

                xt = xt_pool.tile([P, K1 * max_t], bf16, tag="xt", name=f"xt{ci}")
                for o, ln in subs:
                    for k in range(K1):
                        nc.sync.dma_start(xt[:, k * max_t + o:k * max_t + o + ln],
                                          XT[k, :, c0 + o:c0 + o + ln])
                h1 = h1_pool.tile([P, K2 * max_t], bf16, tag="h1", name=f"h1_{ci}")
                yac = y_pool.tile([P, J3 * max_t], f32, tag="ya", name=f"ya{ci}")

                # ---- layer 1: H1 = gelu(X @ W1 + b1), feature-major ----
                # k-outer / sub-inner so consecutive matmuls alternate PSUM
                # banks (same-bank back-to-back serializes drain vs fill).
                for mb in range(MB1):
                    if mb in w1_pre:
                        w1t = w1_pre[mb]
                    else:
                        w1t = w1_pool.tile([P, K1 * P], bf16, tag="w1",
                                           name=f"w1_{ci}_{mb}")
                        nc.sync.dma_start(w1t[:], W1P[mb])
                    ps = {si_: psA.tile([P, 512], f32, tag="ps",
                                        name=f"l1ps{ci}_{mb}_{si_}")
                          for si_ in range(ns)}
                    for k in range(K1):
                        for si_, (o, ln) in enumerate(subs):
                            nc.tensor.matmul(ps[si_][:, :ln],
                                             w1t[:, k * P:(k + 1) * P],
                                             xt[:, k * max_t + o:k * max_t + o + ln],
                                             start=(k == 0), stop=(k == K1 - 1))
                    for si_, (o, ln) in enumerate(subs):
                        nc.scalar.activation(h1[:, mb * max_t + o:mb * max_t + o + ln],
                                             ps[si_][:, :ln], GELU, bias=b1t[:, mb:mb + 1])

                # ---- layer 2 + windowed layer-3 partials ----
                def emit_l3_window(w, h2w, w3w, last=False):
                    first = (w == 0)
                    for pair in range(J3 // 2):
                        for jh in range(2):
                            j = 2 * pair + jh
                            pys = {si_: psY.tile([P, 512], f32, tag="py",
                                                 name=f"py{ci}_{w}_{pair}_{jh}_{si_}")
                                   for si_ in range(ns)}
                            for wi in range(WIN):
                                for si_, (o, ln) in enumerate(subs):
                                    nc.tensor.matmul(
                                        pys[si_][:, :ln],
                                        w3w[wi][:, j * P:(j + 1) * P],
                                        h2w[si_][:, wi * 512:wi * 512 + ln],
                                        start=(wi == 0), stop=(wi == WIN - 1))
                            for si_, (o, ln) in enumerate(subs):
                                dst = yac[:, j * max_t + o:j * max_t + o + ln]
                                if first:
                                    nc.vector.tensor_copy(dst, pys[si_][:, :ln])
                                else:
                                    nc.vector.tensor_add(dst, dst, pys[si_][:, :ln])
                            if last:
                                # evict this j immediately; overlaps the
                                # remaining pairs' matmuls
                                for o, ln in subs:
                                    yv = yev_pool.tile([P, 512], f32, tag="yev")
                                    nc.scalar.activation(
                                        yv[:, :ln],
                                        yac[:, j * max_t + o:j * max_t + o + ln],
                                        IDENT, bias=b3t[:, j:j + 1])
                                    nc.sync.dma_start(
                                        YT[j, :, c0 + o:c0 + o + ln], yv[:, :ln])

                pend = None
                for w in range(MB2 // WIN):
                    w3w = {}
                    h2w = {si_: h2_pool.tile([P, WIN * 512], bf16, tag=f"h2_{si_}",
                                             name=f"h2_{ci}_{w}_{si_}")
                           for si_ in range(ns)}
                    for wi in range(WIN):
                        jj = WIN * w + wi
                        w2t = w2_pool.tile([P, K2 * P], bf16, tag="w2",
                                           name=f"w2_{ci}_{jj}")
                        nc.sync.dma_start(w2t[:], W2P[jj])
                        w3t = w3_pool.tile([P, EMB], bf16, tag="w3", name=f"w3_{ci}_{jj}")
                        nc.sync.dma_start(w3t[:], W3P[jj])
                        w3w[wi] = w3t
                        ps = {si_: psA.tile([P, 512], f32, tag="ps",
                                            name=f"l2ps{ci}_{jj}_{si_}")
                              for si_ in range(ns)}
                        for k in range(K2):
                            for si_, (o, ln) in enumerate(subs):
                                nc.tensor.matmul(ps[si_][:, :ln],
                                                 w2t[:, k * P:(k + 1) * P],
                                                 h1[:, k * max_t + o:k * max_t + o + ln],
                                                 start=(k == 0), stop=(k == K2 - 1))
                        for si_, (o, ln) in enumerate(subs):
                            nc.scalar.activation(h2w[si_][:, wi * 512:wi * 512 + ln],
                                                 ps[si_][:, :ln], GELU, bias=b2t[:, jj:jj + 1])
                    if pend is not None:
                        emit_l3_window(*pend)
                    pend = (w, h2w, w3w)
                emit_l3_window(*pend, last=True)
                c0 += T

    _strip_redundant_sem_incs(nc)
    nc.compile()
    return nc


LAST_RUN = {}


def kernel(x, Wg, bg, W1, b1, W2, b2, W3, b3):
    B, N, E = x.shape
    xf = np.ascontiguousarray(x.reshape(-1, E), dtype=np.float32)

    # ---- host gating (float64 ordering is stable vs the fp32 reference) ----
    s = xf.astype(np.float64) @ Wg.astype(np.float64) + bg.astype(np.float64)
    ti = np.argsort(-s, axis=1, kind="stable")[:, :TOPK]
    tv = np.take_along_axis(s, ti, axis=1)
    ex = np.exp(tv - tv.max(axis=1, keepdims=True))
    gates = (ex / ex.sum(axis=1, keepdims=True)).astype(np.float32)

    idx_e, gate_e = [], []
    for e in range(NE):
        m0 = ti[:, 0] == e
        m1 = ti[:, 1] == e
        idx_e.append(np.concatenate([np.nonzero(m0)[0], np.nonzero(m1)[0]]))
        gate_e.append(np.concatenate([gates[m0, 0], gates[m1, 1]]))
    counts = [len(i) for i in idx_e]

    # The V-slot balance scheme is currently disabled: a V chunk must stream
    # the full 52MB weight set over few tokens, making it weight-DMA-bound;
    # measured against the ~50us capacity saving it is a net loss.
    scheme = None
    if scheme is not None:
        C0, V1, V2, bins = scheme
    else:
        C0, V1, V2 = max(256, -(-max(counts) // 8) * 8), 0, 0
        bins = [(0, 0)] * NE
    C = C0 + V1 + V2

    # ---- slot assignment ----
    # core i main slot: expert i tokens [:C0]; overflow spread over V bins.
    v1_owner = [None] * NE   # per core: (expert, tok_idx, gates) for V1 slot
    v2_owner = [None] * NE
    v1_free = list(range(NE))
    v2_free = list(range(NE))
    for e in range(NE):
        a, bcnt = bins[e]
        rest_i = idx_e[e][C0:]
        rest_g = gate_e[e][C0:]
        pos = 0
        for _ in range(a):
            core = v1_free.pop(0)
            take = min(V1, len(rest_i) - pos)
            v1_owner[core] = (e, rest_i[pos:pos + take], rest_g[pos:pos + take])
            pos += take
        for _ in range(bcnt):
            core = v2_free.pop(0)
            take = min(V2, len(rest_i) - pos)
            v2_owner[core] = (e, rest_i[pos:pos + take], rest_g[pos:pos + take])
            pos += take
        assert pos == len(rest_i), "overflow packing failed"

    # ---- per-expert weight arenas (bf16) ----
    bf = ml_dtypes.bfloat16
    arenas = []
    for e in range(NE):
        arenas.append(dict(
            W1=np.ascontiguousarray(
                W1[e].reshape(K1, P, MB1, P).transpose(2, 1, 0, 3),
                np.float32).reshape(MB1, P, K1 * P).astype(bf),
            W2=np.ascontiguousarray(
                W2[e].reshape(K2, P, MB2, P).transpose(2, 1, 0, 3),
                np.float32).reshape(MB2, P, K2 * P).astype(bf),
            W3=np.ascontiguousarray(W3[e], np.float32).reshape(MB2, P, EMB).astype(bf),
            B1=np.ascontiguousarray(b1[e].reshape(MB1, P).T, np.float32),
            B2=np.ascontiguousarray(b2[e].reshape(MB2, P).T, np.float32),
            B3=np.ascontiguousarray(b3[e].reshape(EMB // P, P).T, np.float32),
        ))

    in_maps = []
    seg_info = []   # per core: list of (col_off, tok_idx, gates)
    for i in range(NE):
        xe = np.zeros((C, EMB), np.float32)
        segs = []
        n_main = min(counts[i], C0)
        xe[:n_main] = xf[idx_e[i][:n_main]]
        segs.append((0, idx_e[i][:n_main], gate_e[i][:n_main]))
        off = C0
        for V, owner in ((V1, v1_owner[i]), (V2, v2_owner[i])):
            if V and owner is not None:
                e_o, t_o, g_o = owner
                xe[off:off + len(t_o)] = xf[t_o]
                segs.append((off, t_o, g_o))
            off += V
        m = {
            "XT": np.ascontiguousarray(xe.T).reshape(K1, P, C).astype(bf),
            "W1A": arenas[i]["W1"], "W2A": arenas[i]["W2"], "W3A": arenas[i]["W3"],
            "B1": arenas[i]["B1"], "B2": arenas[i]["B2"], "B3": arenas[i]["B3"],
        }
        for sfx, V, owner in (("V1", V1, v1_owner[i]), ("V2", V2, v2_owner[i])):
            if not V:
                continue
            e_o = owner[0] if owner is not None else i
            m[f"W1A{sfx}"] = arenas[e_o]["W1"]
            m[f"W2A{sfx}"] = arenas[e_o]["W2"]
            m[f"W3A{sfx}"] = arenas[e_o]["W3"]
            m[f"B1{sfx}"] = arenas[e_o]["B1"]
            m[f"B2{sfx}"] = arenas[e_o]["B2"]
            m[f"B3{sfx}"] = arenas[e_o]["B3"]
        in_maps.append(m)
        seg_info.append(segs)

    trace = bool(int(os.environ.get("KERNEL_TRACE", "0")))
    if trace:
        _install_ntff_hook()
    nc = _build_program(C0, V1, V2)
    res = run_bass_kernel_spmd(nc, in_maps, core_ids=list(range(NE)), trace=trace)
    LAST_RUN["exec_time_ns"] = res.exec_time_ns
    LAST_RUN["capacity"] = C
    LAST_RUN["scheme"] = (C0, V1, V2)

    out = np.zeros_like(xf)
    for i in range(NE):
        yt = res.results[i]["YT"].reshape(EMB, C)
        for off, t_idx, g in seg_info[i]:
            if len(t_idx):
                out[t_idx] += g[:, None] * yt[:, off:off + len(t_idx)].T
    return out.reshape(B, N, E)
